# revision 25
# baseline (speedup 1.0000x reference)
"""Trainium2 Bass kernel for nn_DownsampleBlock (FPS + MLP/BN + attention).

Self-contained. Batch-parallel: 8 batches on 8 NeuronCores (1 each).
Phases per core: MLP (fp32r convs, cross-core BN-stat allreduce) ->
exact-fp32 farthest point sampling (512 sequential iterations) ->
gather + single-head attention (fp32r / bf16).
"""
import sys

sys.path.insert(0, "/opt/trn_rl_repo")

import numpy as np

import concourse.bacc as bacc
import concourse.bass as bass
import concourse.mybir as mybir
from concourse.bass import ds
from concourse.masks import make_identity
from concourse.tile import TileContext

F32 = mybir.dt.float32
F32R = mybir.dt.float32r
BF16 = mybir.dt.bfloat16
I32 = mybir.dt.int32
U16 = mybir.dt.uint16
AF = mybir.ActivationFunctionType
OP = mybir.AluOpType
AX = mybir.AxisListType
SP = mybir.EngineType.SP

BN_EPS = 1e-5
LRELU = 0.2


class CFG:
    def __init__(self, N=8192, M=512, unroll=8, n_cores=8):
        assert N % 1024 == 0 and M % 128 == 0
        self.N = N
        self.M = M
        self.C = 64
        self.NH = N // 2
        self.GS = N // 128
        self.unroll = unroll
        self.n_cores = n_cores


def build_program(cfg: CFG, phases: str = "mfa"):
    nc = bacc.Bacc("TRN2", target_bir_lowering=False, debug=False,
                   num_devices=cfg.n_cores)
    N, M, C, NH, GS = cfg.N, cfg.M, cfg.C, cfg.NH, cfg.GS
    H1, D = 128, 256
    NC512 = N // 512
    NB = N // 128
    MT = M // 128
    core_ids = list(range(cfg.n_cores))
    CNT = float(cfg.n_cores * N)

    def dram_in(name, shape, dtype=F32):
        return nc.dram_tensor(name, shape, dtype, kind="ExternalInput").ap()

    x2 = dram_in("x2", [128, NH])            # support halves-stacked fp32
    supp = dram_in("supp", [C, N])           # support fp32 (conv1 rhs)
    xg = dram_in("xg", [C + 1, N])           # rows 0..63: -2*support; row 64: sq
    sqt = dram_in("sqt", [128, GS])          # sq point-major
    idx0 = dram_in("idx0", [1, 1], I32)
    xyzt = dram_in("xyzt", [N, 4])           # xyz transposed, padded
    w1t = dram_in("w1t", [C, H1])
    w2t = dram_in("w2t", [H1, D])
    w3t = dram_in("w3t", [D, D])
    wqt = dram_in("wqt", [D, D])
    wkt = dram_in("wkt", [D, D])
    wvt = dram_in("wvt", [D, D])
    wot = dram_in("wot", [D, D])
    gb1 = dram_in("gb1", [H1, 2])
    gb2 = dram_in("gb2", [D, 2])
    b3v = dram_in("b3v", [D, 1])

    out_xyzt = nc.dram_tensor("out_xyzt", [M, 4], F32, kind="ExternalOutput").ap()
    out_feat = nc.dram_tensor("out_feat", [D, M], F32, kind="ExternalOutput").ap()
    out_gidx = nc.dram_tensor("out_gidx", [1, M], I32, kind="ExternalOutput").ap()

    ftr_dram = nc.dram_tensor("ftr_scratch", [N, D], F32).ap()
    st1_in = nc.dram_tensor("st1_in", [H1, 2], F32).ap()
    st1_out = nc.dram_tensor("st1_out", [H1, 2], F32).ap()
    st2_in = nc.dram_tensor("st2_in", [D, 2], F32).ap()
    st2_out = nc.dram_tensor("st2_out", [D, 2], F32).ap()

    with TileContext(nc) as tc:
      with tc.tile_pool(name="outer", bufs=1) as po:
        ident = po.tile([128, 128], F32)
        make_identity(nc, ident[:])
        ident_r = po.tile([128, 128], F32R)
        nc.vector.tensor_copy(ident_r[:], ident[:])
        gidx = po.tile([1, M + 1], I32)
        nc.vector.memset(gidx[:], 0)
        nc.sync.dma_start(out=gidx[0:1, 0:1], in_=idx0[:])
        fa = po.tile([128, N], F32R, name="feats_a")
        fb = po.tile([128, N], F32R, name="feats_b")

        # =================== PHASE 1: MLP ===================
        if "m" in phases:
          with tc.tile_pool(name="mlpw", bufs=1) as mw:
            # small persistent-for-phase tiles
            s1 = mw.tile([128, NC512], F32)
            q1 = mw.tile([128, NC512], F32)
            s2 = mw.tile([128, 2 * NC512], F32)
            q2 = mw.tile([128, 2 * NC512], F32)
            scr = mw.tile([128, 512], F32)
            bn1 = mw.tile([H1, 2], F32)
            nc.sync.dma_start(out=bn1[:], in_=gb1[:])
            bn2a = mw.tile([128, 2], F32)
            nc.sync.dma_start(out=bn2a[:], in_=gb2[0:128, :])
            bn2b = mw.tile([128, 2], F32)
            nc.sync.dma_start(out=bn2b[:], in_=gb2[128:256, :])
            b3a = mw.tile([128, 1], F32)
            nc.sync.dma_start(out=b3a[:], in_=b3v[0:128, :])
            b3b = mw.tile([128, 1], F32)
            nc.sync.dma_start(out=b3b[:], in_=b3v[128:256, :])

            wst = mw.tile([128, D], F32, name="wstage")

            def load_round(pool, name, src, p, f):
                nc.sync.dma_start(out=wst[0:p, 0:f], in_=src)
                r = pool.tile([p, f], F32R, name=name + "_r")
                nc.vector.tensor_copy(r[:], wst[0:p, 0:f])
                return r

            w2r = load_round(mw, "w2", w2t[:], H1, D)
            w3ra = load_round(mw, "w3a", w3t[0:128, :], 128, D)
            w3rb = load_round(mw, "w3b", w3t[128:256, :], 128, D)

            def allreduce_stats(parts, d_in, d_out):
                st = mw.tile([128, 2], F32, name=f"st_{parts[0][2]}_{id(parts)%97}",
                             tag="st_stage")
                outs = []
                for (ss, qq, ro, rows) in parts:
                    nc.vector.reduce_sum(st[0:rows, 0:1], ss, axis=AX.X)
                    nc.vector.reduce_sum(st[0:rows, 1:2], qq, axis=AX.X)
                    nc.sync.dma_start(out=d_in[ro:ro + rows, :], in_=st[0:rows, :])
                nc.gpsimd.collective_compute(
                    "AllReduce", OP.add, replica_groups=[core_ids],
                    ins=[d_in[:]], outs=[d_out[:]])
                for (ss, qq, ro, rows) in parts:
                    g = mw.tile([128, 2], F32, name=f"stg_{ro}_{id(parts)%97}")
                    nc.sync.dma_start(out=g[0:rows, :], in_=d_out[ro:ro + rows, :])
                    outs.append(g)
                return outs

            def bn_affine(stats, gbt, rows, name):
                inv = 1.0 / CNT
                m = mw.tile([128, 1], F32, name=name + "_m")
                nc.vector.tensor_scalar(m[0:rows], stats[0:rows, 0:1], inv, None,
                                        op0=OP.mult)
                v = mw.tile([128, 1], F32, name=name + "_v")
                nc.vector.tensor_scalar(v[0:rows], stats[0:rows, 1:2], inv, None,
                                        op0=OP.mult)
                mm_ = mw.tile([128, 1], F32, name=name + "_mm")
                nc.vector.tensor_tensor(out=mm_[0:rows], in0=m[0:rows],
                                        in1=m[0:rows], op=OP.mult)
                nc.vector.tensor_tensor(out=v[0:rows], in0=v[0:rows],
                                        in1=mm_[0:rows], op=OP.subtract)
                nc.vector.tensor_scalar(v[0:rows], v[0:rows], BN_EPS, None,
                                        op0=OP.add)
                r = mw.tile([128, 1], F32, name=name + "_r")
                nc.vector.reciprocal(r[0:rows], v[0:rows])
                rs = mw.tile([128, 1], F32, name=name + "_rs")
                nc.scalar.sqrt(rs[0:rows], r[0:rows])
                sc = mw.tile([128, 1], F32, name=name + "_sc")
                nc.vector.tensor_tensor(out=sc[0:rows], in0=rs[0:rows],
                                        in1=gbt[0:rows, 0:1], op=OP.mult)
                bi = mw.tile([128, 1], F32, name=name + "_bi")
                nc.vector.tensor_tensor(out=bi[0:rows], in0=m[0:rows],
                                        in1=sc[0:rows], op=OP.mult)
                nc.vector.tensor_tensor(out=bi[0:rows], in0=gbt[0:rows, 1:2],
                                        in1=bi[0:rows], op=OP.subtract)
                return sc, bi

            with tc.tile_pool(name="mlp_y1", bufs=1) as my1, \
                 tc.tile_pool(name="mlp_ps", bufs=2, space="PSUM") as mps:
                y1 = my1.tile([128, N], F32R)
                # ---- conv1 (fp32) ----
                with tc.tile_pool(name="mlp_c1", bufs=1) as mc1:
                    supp_sb = mc1.tile([C, N], F32)
                    nc.sync.dma_start(out=supp_sb[:], in_=supp[:])
                    w1s = mc1.tile([C, H1], F32)
                    nc.sync.dma_start(out=w1s[:], in_=w1t[:])
                    h1 = mc1.tile([128, N], F32)
                    for k in range(NC512):
                        ps = mps.tile([128, 512], F32, tag="cps", name=f"c1_{k}")
                        nc.tensor.matmul(out=ps[:], lhsT=w1s[:],
                                         rhs=supp_sb[:, ds(k * 512, 512)],
                                         start=True, stop=True)
                        nc.scalar.activation(h1[:, ds(k * 512, 512)], ps[:],
                                             AF.Identity,
                                             accum_out=s1[:, k:k + 1])
                        nc.scalar.activation(scr[:], ps[:], AF.Square,
                                             accum_out=q1[:, k:k + 1])
                    g1 = allreduce_stats([(s1[:], q1[:], 0, H1)],
                                         st1_in, st1_out)[0]
                    sc1, bi1 = bn_affine(g1, bn1, H1, "bn1")
                    nc.scalar.activation(y1[:], h1[:], AF.Identity,
                                         bias=bi1[:, 0:1], scale=sc1[:, 0:1])
                    nc.vector.scalar_tensor_tensor(
                        out=y1[:], in0=y1[:], scalar=LRELU, in1=y1[:],
                        op0=OP.mult, op1=OP.max)

                # ---- conv2 (fp32r) + in-place bn/lrelu ----
                with tc.tile_pool(name="mlp_h2", bufs=1) as mh2:
                    h2a = mh2.tile([128, N], F32R)
                    h2b = mh2.tile([128, N], F32R)
                    for mh, h2t in ((0, h2a), (1, h2b)):
                        for k in range(NC512):
                            ps = mps.tile([128, 512], F32, tag="cps",
                                          name=f"c2_{mh}_{k}")
                            nc.tensor.matmul(out=ps[:],
                                             lhsT=w2r[:, ds(mh * 128, 128)],
                                             rhs=y1[:, ds(k * 512, 512)],
                                             start=True, stop=True)
                            col = mh * NC512 + k
                            nc.scalar.activation(h2t[:, ds(k * 512, 512)], ps[:],
                                                 AF.Identity,
                                                 accum_out=s2[:, col:col + 1])
                            nc.scalar.activation(scr[:], ps[:], AF.Square,
                                                 accum_out=q2[:, col:col + 1])
                    g2a, g2b = allreduce_stats(
                        [(s2[:, 0:NC512], q2[:, 0:NC512], 0, 128),
                         (s2[:, NC512:], q2[:, NC512:], 128, 128)],
                        st2_in, st2_out)
                    sc2a, bi2a = bn_affine(g2a, bn2a, 128, "bn2a")
                    sc2b, bi2b = bn_affine(g2b, bn2b, 128, "bn2b")
                    for h2t, sc, bi in ((h2a, sc2a, bi2a), (h2b, sc2b, bi2b)):
                        nc.scalar.activation(h2t[:], h2t[:], AF.Identity,
                                             bias=bi[:, 0:1], scale=sc[:, 0:1])
                        nc.vector.scalar_tensor_tensor(
                            out=h2t[:], in0=h2t[:], scalar=LRELU, in1=h2t[:],
                            op0=OP.mult, op1=OP.max)

                    # ---- conv3 (fp32r) -> feats + featsT(dram) ----
                    for mh, ft, b3 in ((0, fa, b3a), (1, fb, b3b)):
                        for k in range(NC512):
                            ps = mps.tile([128, 512], F32, tag="cps",
                                          name=f"c3_{mh}_{k}")
                            nc.tensor.matmul(out=ps[:],
                                             lhsT=w3ra[:, ds(mh * 128, 128)],
                                             rhs=h2a[:, ds(k * 512, 512)],
                                             start=True, stop=False)
                            nc.tensor.matmul(out=ps[:],
                                             lhsT=w3rb[:, ds(mh * 128, 128)],
                                             rhs=h2b[:, ds(k * 512, 512)],
                                             start=False, stop=True)
                            nc.scalar.activation(ft[:, ds(k * 512, 512)], ps[:],
                                                 AF.Identity, bias=b3[:, 0:1])
                # featsT -> DRAM via PE block transposes
                for nb in range(NB):
                    for mh, ft in ((0, fa), (1, fb)):
                        ptp = mps.tile([128, 128], F32R, tag="cps",
                                       name=f"tp{nb}_{mh}")
                        nc.tensor.matmul(out=ptp[:],
                                         lhsT=ft[:, ds(nb * 128, 128)],
                                         rhs=ident_r[:], is_transpose=True,
                                         start=True, stop=True)
                        nc.vector.tensor_copy(wst[:, ds(mh * 128, 128)], ptp[:])
                    nc.sync.dma_start(out=ftr_dram[ds(nb * 128, 128), :],
                                      in_=wst[:])

        # =================== PHASE 2: FPS ===================
        if "f" in phases:
          with tc.tile_pool(name="fps", bufs=1) as fp, \
               tc.tile_pool(name="fps_ps", bufs=2, space="PSUM") as fps_ps:
            x2_sb = fp.tile([128, NH], F32)
            nc.sync.dma_start(out=x2_sb[:], in_=x2[:])
            sqt_sb = fp.tile([128, GS], F32)
            nc.sync.dma_start(out=sqt_sb[:], in_=sqt[:])
            ccol_i = fp.tile([128, 1], I32)
            nc.gpsimd.iota(ccol_i[:], pattern=[[0, 1]], base=N,
                           channel_multiplier=-GS)
            ccol = fp.tile([128, 1], F32)
            nc.vector.tensor_copy(ccol[:], ccol_i[:])

            dpp = fp.tile([128, GS], F32)
            nc.vector.memset(dpp[:], 1e10)
            cent2 = fp.tile([128, 2], F32)
            nc.vector.memset(cent2[:], 0.0)
            csq2 = fp.tile([2, 1], F32)
            drow2 = fp.tile([2, NH], F32)
            dtile = fp.tile([128, GS], F32)
            dscr = fp.tile([128, GS], F32)
            combo = fp.tile([128, 2], F32)
            rowmax_t = fp.tile([128, 1], F32)
            iw8 = fp.tile([128, 8], U16)
            rows_a = fp.tile([1, 128], F32)
            rows_b = fp.tile([1, 128], F32)
            v8 = fp.tile([1, 8], F32)
            oh = fp.tile([1, 128], F32)
            trash = fp.tile([1, 128], F32)
            nmax = fp.tile([1, 1], F32)

            CW = min(512, NH)
            NCH = NH // CW

            import os
            FPARTS = os.environ.get("FPS_PARTS", "full")

            def fps_body(iv):
                off = nc.values_load(gidx[0:1, ds(iv, 1)], engines=(SP,),
                                     min_val=0, max_val=N - 1,
                                     skip_runtime_bounds_check=True)
                nc.sync.dma_start(out=cent2[0:C, 0:1], in_=xg[0:C, ds(off, 1)])
                nc.sync.dma_start(out=cent2[C:2 * C, 1:2],
                                  in_=xg[0:C, ds(off, 1)])
                nc.sync.dma_start(
                    out=csq2[:, :],
                    in_=xg[C:C + 1, ds(off, 1)].partition_broadcast(2))
                if FPARTS == "g":
                    return
                for k in range(NCH):
                    pd = fps_ps.tile([2, CW], F32, tag="pd", name=f"pd{k}")
                    nc.tensor.matmul(out=pd[:], lhsT=cent2[:],
                                     rhs=x2_sb[:, ds(k * CW, CW)],
                                     start=True, stop=True)
                    nc.scalar.activation(drow2[:, ds(k * CW, CW)], pd[:],
                                         AF.Identity, bias=csq2[:, 0:1])
                if FPARTS == "gm":
                    return
                nc.sync.dma_start(out=dtile[:, :], in_=drow2[:, :])
                if FPARTS == "gms":
                    return
                nc.vector.tensor_tensor(out=dpp[:], in0=dpp[:], in1=dtile[:],
                                        op=OP.min)
                if FPARTS == "d1":
                    return
                nc.vector.tensor_tensor(out=dscr[:], in0=dpp[:],
                                        in1=sqt_sb[:], op=OP.add)
                nc.vector.tensor_reduce(out=combo[:, 0:1], in_=dscr[:],
                                        axis=AX.X, op=OP.max)
                if FPARTS in ("d2", "d2b"):
                    return
                nc.vector.max_index(out=iw8[:],
                                    in_max=combo[:, 0:1].to_broadcast([128, 8]),
                                    in_values=dscr[:])
                if FPARTS == "d3":
                    return
                nc.vector.tensor_scalar(combo[:, 1:2], iw8[:, 0:1], -1.0,
                                        ccol[:, 0:1], op0=OP.mult, op1=OP.add)
                if FPARTS == "gmsd":
                    return
                pta = fps_ps.tile([1, 128], F32, tag="pd", name="pta")
                nc.tensor.matmul(out=pta[:], lhsT=combo[:, 0:1], rhs=ident[:],
                                 is_transpose=True, start=True, stop=True)
                ptb = fps_ps.tile([1, 128], F32, tag="pd", name="ptb")
                nc.tensor.matmul(out=ptb[:], lhsT=combo[:, 1:2], rhs=ident[:],
                                 is_transpose=True, start=True, stop=True)
                nc.vector.tensor_copy(rows_a[:], pta[:])
                nc.vector.tensor_copy(rows_b[:], ptb[:])
                nc.vector.max(out=v8[:], in_=rows_a[:])
                nc.vector.tensor_scalar(oh[:], rows_a[:], v8[0:1, 0:1],
                                        None, op0=OP.is_equal)
                nc.vector.tensor_tensor(out=trash[:], in0=oh[:], in1=rows_b[:],
                                        op=OP.mult)
                nc.vector.tensor_reduce(out=nmax[:], in_=trash[:],
                                        axis=AX.X, op=OP.max)
                nc.vector.tensor_scalar(gidx[0:1, ds(iv + 1, 1)], nmax[:],
                                        -1.0, float(N), op0=OP.mult, op1=OP.add)

            tc.For_i_unrolled(0, M, 1, fps_body, max_unroll=cfg.unroll)
            nc.sync.dma_start(out=out_gidx[:], in_=gidx[0:1, 0:M])

        # =================== PHASE 3: gather + attention ===================
        if "a" in phases:
          with tc.tile_pool(name="att", bufs=1) as ap_, \
               tc.tile_pool(name="att_ps", bufs=2, space="PSUM") as aps, \
               tc.tile_pool(name="att_ps1", bufs=1, space="PSUM") as aps1:
            ca = ap_.tile([128, M], F32)
            cb = ap_.tile([128, M], F32)
            with tc.tile_pool(name="gat", bufs=1) as gp:
                # idxc[p, t] = centroid (p*MT + t); column order is un-permuted
                # host-side (out_feat/out_xyzt are column/row permuted).
                idxc = gp.tile([128, MT], I32)
                nc.sync.dma_start(out=idxc[:], in_=gidx[0:1, 0:M])
                xyzg = gp.tile([128, 4], F32)
                centT = gp.tile([128, MT * D], F32)
                for t in range(MT):
                    nc.gpsimd.indirect_dma_start(
                        out=centT[:, ds(t * D, D)], out_offset=None,
                        in_=ftr_dram[:],
                        in_offset=bass.IndirectOffsetOnAxis(
                            ap=idxc[:, t:t + 1], axis=0))
                    nc.gpsimd.indirect_dma_start(
                        out=xyzg[:], out_offset=None, in_=xyzt[:],
                        in_offset=bass.IndirectOffsetOnAxis(
                            ap=idxc[:, t:t + 1], axis=0))
                    nc.sync.dma_start(out=out_xyzt[ds(t * 128, 128), :],
                                      in_=xyzg[:])
                for t in range(MT):
                    for mh, ct in ((0, ca), (1, cb)):
                        ptc = aps.tile([128, 128], F32, tag="aps",
                                       name=f"ct{t}{mh}")
                        nc.tensor.matmul(
                            out=ptc[:],
                            lhsT=centT[:, ds(t * D + mh * 128, 128)],
                            rhs=ident[:], is_transpose=True,
                            start=True, stop=True)
                        nc.vector.tensor_copy(ct[:, ds(t * 128, 128)], ptc[:])
            car = ap_.tile([128, M], F32R)
            nc.vector.tensor_copy(car[:], ca[:])
            cbr = ap_.tile([128, M], F32R)
            nc.vector.tensor_copy(cbr[:], cb[:])

            wst2 = ap_.tile([128, D], F32, name="wstage2")

            def load_round2(name, src, f):
                nc.sync.dma_start(out=wst2[:, 0:f], in_=src)
                r = ap_.tile([128, f], F32R, name=name + "_r")
                nc.vector.tensor_copy(r[:], wst2[:, 0:f])
                return r

            wqra = load_round2("wqa", wqt[0:128, :], D)
            wqrb = load_round2("wqb", wqt[128:256, :], D)
            wkra = load_round2("wka", wkt[0:128, :], D)
            wkrb = load_round2("wkb", wkt[128:256, :], D)
            wvra = load_round2("wva", wvt[0:128, :], D)
            wvrb = load_round2("wvb", wvt[128:256, :], D)
            wora = load_round2("woa", wot[0:128, :], D)
            worb = load_round2("wob", wot[128:256, :], D)

            qa = ap_.tile([128, M], F32R)
            qb = ap_.tile([128, M], F32R)
            for mh, qt in ((0, qa), (1, qb)):
                ps = aps.tile([128, M], F32, tag="aps", name=f"q{mh}")
                nc.tensor.matmul(out=ps[:], lhsT=wqra[:, ds(mh * 128, 128)],
                                 rhs=car[:], start=True, stop=False)
                nc.tensor.matmul(out=ps[:], lhsT=wqrb[:, ds(mh * 128, 128)],
                                 rhs=cbr[:], start=False, stop=True)
                nc.scalar.activation(qt[:], ps[:], AF.Identity)

            ka = ap_.tile([128, N], F32R)
            kb = ap_.tile([128, N], F32R)
            for mh, kt in ((0, ka), (1, kb)):
                for kk in range(NC512):
                    ps = aps.tile([128, 512], F32, tag="aps", name=f"k{mh}_{kk}")
                    nc.tensor.matmul(out=ps[:],
                                     lhsT=wkra[:, ds(mh * 128, 128)],
                                     rhs=fa[:, ds(kk * 512, 512)],
                                     start=True, stop=False)
                    nc.tensor.matmul(out=ps[:],
                                     lhsT=wkrb[:, ds(mh * 128, 128)],
                                     rhs=fb[:, ds(kk * 512, 512)],
                                     start=False, stop=True)
                    nc.scalar.activation(kt[:, ds(kk * 512, 512)], ps[:],
                                         AF.Identity)

            vT = ap_.tile([128, NB * D], BF16)
            for nb in range(NB):
                ps = aps.tile([128, D], F32, tag="aps", name=f"v{nb}")
                nc.tensor.matmul(out=ps[:], lhsT=fa[:, ds(nb * 128, 128)],
                                 rhs=wvra[:], start=True, stop=False)
                nc.tensor.matmul(out=ps[:], lhsT=fb[:, ds(nb * 128, 128)],
                                 rhs=wvrb[:], start=False, stop=True)
                nc.scalar.activation(vT[:, ds(nb * D, D)], ps[:], AF.Identity)

            att0 = aps1.tile([128, M], F32, name="att0")
            att1 = aps1.tile([128, M], F32, name="att1")
            sig = aps1.tile([1, M], F32, name="sig")
            ones_bf = ap_.tile([128, 1], BF16)
            nc.vector.memset(ones_bf[:], 1.0)
            with tc.tile_pool(name="probs", bufs=2) as pb:
                for nb in range(NB):
                    psl = aps.tile([128, M], F32, tag="aps", name=f"l{nb}")
                    nc.tensor.matmul(out=psl[:], lhsT=ka[:, ds(nb * 128, 128)],
                                     rhs=qa[:], start=True, stop=False)
                    nc.tensor.matmul(out=psl[:], lhsT=kb[:, ds(nb * 128, 128)],
                                     rhs=qb[:], start=False, stop=True)
                    prob = pb.tile([128, M], BF16, tag="pr", name=f"p{nb}")
                    nc.scalar.activation(prob[:], psl[:], AF.Exp,
                                         scale=1.0 / 16.0)
                    nc.tensor.matmul(out=sig[:], lhsT=ones_bf[:], rhs=prob[:],
                                     start=(nb == 0), stop=(nb == NB - 1))
                    nc.tensor.matmul(out=att0[:], lhsT=vT[:, ds(nb * D, 128)],
                                     rhs=prob[:], start=(nb == 0),
                                     stop=(nb == NB - 1))
                    nc.tensor.matmul(out=att1[:],
                                     lhsT=vT[:, ds(nb * D + 128, 128)],
                                     rhs=prob[:], start=(nb == 0),
                                     stop=(nb == NB - 1))

            srow = ap_.tile([1, M], F32)
            nc.vector.tensor_copy(srow[:], sig[:])
            sinv = ap_.tile([1, M], F32)
            nc.vector.reciprocal(sinv[:], srow[:])
            sinr = ap_.tile([1, M], F32R)
            nc.vector.tensor_copy(sinr[:], sinv[:])
            ones_f = ap_.tile([1, 128], F32)
            nc.vector.memset(ones_f[:], 1.0)
            ones_r = ap_.tile([1, 128], F32R)
            nc.vector.tensor_copy(ones_r[:], ones_f[:])
            bcast = aps1.tile([128, M], F32, name="bcast")
            nc.tensor.matmul(out=bcast[:], lhsT=ones_r[:], rhs=sinr[:],
                             start=True, stop=True)
            bsb = ap_.tile([128, M], F32)
            nc.scalar.activation(bsb[:], bcast[:], AF.Identity)
            an0 = ap_.tile([128, M], F32R)
            an1 = ap_.tile([128, M], F32R)
            nc.vector.tensor_tensor(out=an0[:], in0=att0[:], in1=bsb[:],
                                    op=OP.mult)
            nc.vector.tensor_tensor(out=an1[:], in0=att1[:], in1=bsb[:],
                                    op=OP.mult)
            for mh, ct in ((0, ca), (1, cb)):
                ps = aps.tile([128, M], F32, tag="aps", name=f"o{mh}")
                nc.tensor.matmul(out=ps[:], lhsT=wora[:, ds(mh * 128, 128)],
                                 rhs=an0[:], start=True, stop=False)
                nc.tensor.matmul(out=ps[:], lhsT=worb[:, ds(mh * 128, 128)],
                                 rhs=an1[:], start=False, stop=True)
                o = ap_.tile([128, M], F32, name=f"out{mh}")
                nc.vector.tensor_tensor(out=o[:], in0=ps[:], in1=ct[:],
                                        op=OP.add)
                nc.sync.dma_start(out=out_feat[ds(mh * 128, 128), :], in_=o[:])

    nc.compile()
    return nc


# ---------------------------------------------------------------------------
# Host side
# ---------------------------------------------------------------------------
_CACHE = {}


def prep_core_inputs(cfg, support, xyz_b, far0):
    N = cfg.N
    sq = (support.astype(np.float32) ** 2).sum(0, dtype=np.float32)
    x2 = np.concatenate([support[:, :N // 2], support[:, N // 2:]], axis=0)
    xg = np.concatenate([-2.0 * support, sq[None, :]], axis=0)
    xyzt = np.zeros((N, 4), dtype=np.float32)
    xyzt[:, :3] = xyz_b.T
    return {
        "x2": np.ascontiguousarray(x2, np.float32),
        "supp": np.ascontiguousarray(support, np.float32),
        "xg": np.ascontiguousarray(xg, np.float32),
        "sqt": np.ascontiguousarray(sq.reshape(128, cfg.GS), np.float32),
        "idx0": np.array([[far0]], dtype=np.int32),
        "xyzt": xyzt,
    }


def shared_inputs(w1, w2, w3, wq, wk, wv, wo, g1, be1, g2, be2, b3):
    f32 = lambda a: np.ascontiguousarray(np.asarray(a, np.float32))
    return {
        "w1t": f32(np.asarray(w1).T), "w2t": f32(np.asarray(w2).T),
        "w3t": f32(np.asarray(w3).T), "wqt": f32(np.asarray(wq).T),
        "wkt": f32(np.asarray(wk).T), "wvt": f32(np.asarray(wv).T),
        "wot": f32(np.asarray(wo).T),
        "gb1": f32(np.stack([np.asarray(g1), np.asarray(be1)], 1)),
        "gb2": f32(np.stack([np.asarray(g2), np.asarray(be2)], 1)),
        "b3v": f32(np.asarray(b3).reshape(-1, 1)),
    }


def unpermute_outputs(cfg, ox, of):
    M, MT = cfg.M, cfg.M // 128
    ox = ox.reshape(MT, 128, 4).transpose(1, 0, 2).reshape(M, 4)
    of = of.reshape(-1, MT, 128).transpose(0, 2, 1).reshape(-1, M)
    return ox, of


def kernel(xyz, point_features, w1, b1, g1, be1, w2, b2, g2, be2, w3, b3,
           wq, wk, wv, wo, farthest_init):
    from concourse.bass_utils import run_bass_kernel_spmd

    cfg = CFG()
    if "nc" not in _CACHE:
        _CACHE["nc"] = build_program(cfg)
    nc = _CACHE["nc"]

    xyz = np.asarray(xyz, dtype=np.float32)
    pf = np.asarray(point_features, dtype=np.float32)
    far = np.asarray(farthest_init).astype(np.int32).reshape(-1)
    B = xyz.shape[0]
    support = np.concatenate([xyz, pf], axis=1).astype(np.float32)

    shared = shared_inputs(w1, w2, w3, wq, wk, wv, wo, g1, be1, g2, be2, b3)
    in_maps = []
    for b in range(B):
        m = prep_core_inputs(cfg, support[b], xyz[b], int(far[b]))
        m.update(shared)
        in_maps.append(m)

    res = run_bass_kernel_spmd(nc, in_maps, core_ids=list(range(cfg.n_cores)),
                               **_CACHE.get("run_kwargs", {}))
    _CACHE["last_res"] = res
    new_xyz, feats = [], []
    for b in range(B):
        ox, of = unpermute_outputs(cfg, res.results[b]["out_xyzt"],
                                   res.results[b]["out_feat"])
        new_xyz.append(ox[:, :3].T)
        feats.append(of)
    return (np.stack(new_xyz).astype(np.float32),
            np.stack(feats).astype(np.float32))
